# revision 23
# baseline (speedup 1.0000x reference)
"""3-layer GCN forward pass on 8 TRN2 NeuronCores.

Strategy (vertex-cut graph parallelism):
  - Each core owns a contiguous block of N/8 destination nodes; edges are
    partitioned by destination so segment sums stay local.
  - Per layer, each GCN conv is computed as (A_hat @ x) @ W + b, i.e.
    aggregate first, then the dense matmul.
  - Prologue: x_tilde = dinv * x is computed per-core on its own slice and
    AllGathered; every layer then gathers x_tilde rows per edge with
    dma_gather (4 SWDGE queues in parallel) and segment-sums them on the
    tensor engine as one-hot matmuls accumulating z^T in PSUM (channels on
    partitions).  One-hot S matrices depend only on the graph; they are
    precomputed host-side and streamed from HBM.
  - The destination-side dinv is applied per-column after aggregation;
    self loops are plain edges, reproducing the h/deg self term exactly.
  - Between layers: the per-core slice of the new features is scaled,
    transposed back to node-major (DMA transpose) and AllGathered.
  - int16 gather indices only reach 32768 rows, so each core's edges are
    split into a "lo" stream (padded row id < 32768) and a "hi" stream,
    gathered from two base offsets; PSUM partials from the two passes are
    combined in SBUF.
"""

import sys
import types

sys.path.insert(0, "/opt/trn_rl_repo")

import numpy as np
import ml_dtypes

import concourse.bass as bass  # noqa: F401
import concourse.bacc as bacc
import concourse.tile as tile
import concourse.mybir as mybir
from concourse import bass_utils
from concourse.masks import make_identity

BF16 = ml_dtypes.bfloat16
FP32 = np.float32


def _install_ntff_hook():
    """The image's antenv lacks axon_hooks; shim it so trace=True works."""
    if "antenv.axon_hooks" in sys.modules:
        return
    mod = types.ModuleType("antenv.axon_hooks")
    mod._hook = None
    mod.set_axon_ntff_profile_hook = lambda h: setattr(mod, "_hook", h)
    mod.get_axon_ntff_profile_hook = lambda: mod._hook
    sys.modules["antenv.axon_hooks"] = mod
    try:
        import antenv

        antenv.axon_hooks = mod
        if "/root/.axon_site" not in sys.path:
            sys.path.insert(0, "/root/.axon_site")
        from trn_agent_boot.trn_boot import _ntff_profile_via_ctypes

        mod.set_axon_ntff_profile_hook(
            _ntff_profile_via_ctypes("/opt/axon/libaxon_pjrt.so"))
    except Exception:
        pass


class Cfg:
    def __init__(self, n=50000, c=128, hid=128, out_c=64, ncores=8,
                 lo_rows=32768, piece_ch=4, sbatch=16):
        self.N = n
        self.C = c
        self.HID = hid
        self.OUT_C = out_c
        self.NCORES = ncores
        self.NPC = n // ncores
        self.TPC = (self.NPC + 127) // 128
        self.NPC_PAD = self.TPC * 128
        self.NPAD = ncores * self.NPC_PAD
        self.LO_ROWS = lo_rows
        self.TA = (self.TPC + 1) // 2          # tiles in half A
        self.SPLITA = self.TA * 128            # per-rank rows in half A
        self.ROWSA = ncores * self.SPLITA
        self.ROWSB = ncores * (self.NPC_PAD - self.SPLITA)
        assert self.ROWSA <= 32768 and self.ROWSB <= 32768, \
            "half-buffers must stay int16-addressable"
        self.PIECE_CH = piece_ch
        self.SBATCH = sbatch
        assert n % ncores == 0


FULL = Cfg()


# ---------------------------------------------------------------- host prep
def _preprocess(cfg, edge_index):
    """Compute per-core gather indices and one-hot S blocks."""
    src = np.asarray(edge_index[0]).astype(np.int64)
    dst = np.asarray(edge_index[1]).astype(np.int64)
    n, npc, npcp, tpc = cfg.N, cfg.NPC, cfg.NPC_PAD, cfg.TPC

    deg = (np.bincount(dst, minlength=n) + 1.0).astype(np.float64)
    dinv = (1.0 / np.sqrt(deg)).astype(np.float32)

    loop = np.arange(n, dtype=np.int64)
    allsrc = np.concatenate([src, loop])
    alldst = np.concatenate([dst, loop])

    src_r = allsrc // npc
    src_i = allsrc % npc
    bucket = (src_i >= cfg.SPLITA).astype(np.int64)
    szB = npcp - cfg.SPLITA
    m = np.where(bucket == 0, src_r * cfg.SPLITA + src_i,
                 src_r * szB + (src_i - cfg.SPLITA))
    owner = alldst // npc
    local = alldst - owner * npc
    tile_id = local // 128
    pos = local % 128

    order = np.lexsort((m, bucket, tile_id, owner))
    m_s = m[order]
    owner_s = owner[order]
    tile_s = tile_id[order]
    bucket_s = bucket[order]
    pos_s = pos[order]

    counts = np.zeros((cfg.NCORES, tpc, 2), dtype=np.int64)
    np.add.at(counts, (owner_s, tile_s, bucket_s), 1)
    clo = np.ceil(counts[:, :, 0] / 128).astype(int).max(axis=0)  # [tpc]
    chi = np.ceil(counts[:, :, 1] / 128).astype(int).max(axis=0)

    flat_counts = counts.reshape(-1)
    flat_starts = np.concatenate([[0], np.cumsum(flat_counts)[:-1]])
    starts = flat_starts.reshape(cfg.NCORES, tpc, 2)

    nch = int(clo.sum() + chi.sum())
    l_lo = max(int(clo.sum()) * 128, 16)
    l_hi = max(int(chi.sum()) * 128, 16)

    col_ids = np.arange(128, dtype=np.int32)
    per_core = []
    for c in range(cfg.NCORES):
        idx_lo = np.zeros(l_lo, dtype=np.int32)
        idx_hi = np.zeros(l_hi, dtype=np.int32)
        dpos = np.full((nch, 128), 255, dtype=np.int32)

        ch_g = 0
        for b, cc, idx_arr in ((0, clo, idx_lo), (1, chi, idx_hi)):
            off = 0
            for t in range(tpc):
                s0 = starts[c, t, b]
                cnt = counts[c, t, b]
                cap = cc[t] * 128
                idx_arr[off:off + cnt] = m_s[s0:s0 + cnt]
                blk = dpos[ch_g:ch_g + cc[t]].reshape(-1)
                blk[:cnt] = pos_s[s0:s0 + cnt]
                off += cap
                ch_g += cc[t]
        assert ch_g == nch

        def wrap(stream):
            # idx j consumed from [j % 16, j // 16]; replicate to 8 groups
            a = stream.reshape(-1, 16).T.astype(np.int16)
            return np.tile(a, (8, 1)).copy()

        nblk_s = (nch + cfg.SBATCH - 1) // cfg.SBATCH
        dpos_pad = np.full((nblk_s * cfg.SBATCH, 128), 255, dtype=np.int32)
        dpos_pad[:nch] = dpos
        s_all = (dpos_pad[:, :, None] == col_ids[None, None, :]).astype(BF16)
        # block-contiguous layout: [nblk, 128 part, SBATCH, 128]
        s_all = s_all.reshape(nblk_s, cfg.SBATCH, 128, 128) \
            .transpose(0, 2, 1, 3).reshape(nblk_s * 128, cfg.SBATCH * 128)

        dinv_own = np.zeros(npcp, dtype=np.float32)
        dinv_own[:npc] = dinv[c * npc:(c + 1) * npc]
        per_core.append({
            "idx_lo": wrap(idx_lo),
            "idx_hi": wrap(idx_hi),
            "s_all": np.ascontiguousarray(s_all),
            "dinv_bc": np.tile(dinv_own, (128, 1)).astype(BF16),
            "dinv_nm": np.ascontiguousarray(
                dinv_own.reshape(tpc, 128).T).astype(np.float32),
        })

    plan = {
        "clo": clo.tolist(),
        "chi": chi.tolist(),
        "nch": nch,
        "nblk_s": nblk_s,
        "l_lo": l_lo,
        "l_hi": l_hi,
    }
    return plan, per_core, dinv


# ------------------------------------------------------------- bass program
def _build(cfg, plan):
    clo, chi = plan["clo"], plan["chi"]
    nch, l_lo, l_hi = plan["nch"], plan["l_lo"], plan["l_hi"]
    tpc, npcp, npad = cfg.TPC, cfg.NPC_PAD, cfg.NPAD
    C, OUT_C = cfg.C, cfg.OUT_C
    bf = mybir.dt.bfloat16
    f32 = mybir.dt.float32

    nc = bacc.Bacc("TRN2", target_bir_lowering=False, debug=False,
                   num_devices=cfg.NCORES, num_swdge_queues=4,
                   dynamic_dma_scratch_size=32768)

    xt0_d = nc.dram_tensor("xt0", [npcp, C], bf, kind="ExternalInput")
    w_d = [nc.dram_tensor(f"w{i}", [C, C if i < 2 else OUT_C], bf,
                          kind="ExternalInput") for i in range(3)]
    b_d = [nc.dram_tensor(f"b{i}", [C if i < 2 else OUT_C, 1], f32,
                          kind="ExternalInput") for i in range(3)]
    idxlo_d = nc.dram_tensor("idx_lo", [128, max(l_lo // 16, 1)],
                             mybir.dt.int16, kind="ExternalInput")
    idxhi_d = nc.dram_tensor("idx_hi", [128, max(l_hi // 16, 1)],
                             mybir.dt.int16, kind="ExternalInput")
    nblk_s = plan["nblk_s"]
    s_d = nc.dram_tensor("s_all", [nblk_s * 128, cfg.SBATCH * 128], bf,
                         kind="ExternalInput")
    dinvbc_d = nc.dram_tensor("dinv_bc", [128, npcp], bf,
                              kind="ExternalInput")
    dinvnm_d = nc.dram_tensor("dinv_nm", [128, tpc], f32,
                              kind="ExternalInput")
    out_d = nc.dram_tensor("out", [cfg.NPC, OUT_C], f32,
                           kind="ExternalOutput")

    with tile.TileContext(nc) as tc:
        with (
            tc.tile_pool(name="const", bufs=1) as cpool,
            tc.tile_pool(name="g", bufs=8) as gpool,
            tc.tile_pool(name="s", bufs=5) as spool,
            tc.tile_pool(name="z", bufs=13) as zpool,
            tc.tile_pool(name="zs", bufs=2) as zspool,
            tc.tile_pool(name="xt", bufs=1) as xtpool,
            tc.tile_pool(name="nm", bufs=1) as nmpool,
            tc.tile_pool(name="fin", bufs=1) as finpool,
            tc.tile_pool(name="psA", bufs=5, space="PSUM") as psa,
            tc.tile_pool(name="psW", bufs=2, space="PSUM") as psw_pool,
            tc.tile_pool(name="psT", bufs=1, space="PSUM") as pst,
            tc.tile_pool(name="dram", bufs=1, space="DRAM") as dpool,
        ):
            # ---- constants into SBUF
            w_sb, b_sb = [], []
            for i in range(3):
                w = cpool.tile([C, C if i < 2 else OUT_C], bf, name=f"wt{i}")
                nc.sync.dma_start(w[:], w_d[i][:])
                bt = cpool.tile([C if i < 2 else OUT_C, 1], f32,
                                name=f"bt{i}")
                nc.sync.dma_start(bt[:], b_d[i][:])
                w_sb.append(w)
                b_sb.append(bt)
            idxlo_sb = cpool.tile([128, max(l_lo // 16, 1)], mybir.dt.int16,
                                  tag="idxlo")
            nc.sync.dma_start(idxlo_sb[:], idxlo_d[:])
            idxhi_sb = cpool.tile([128, max(l_hi // 16, 1)], mybir.dt.int16,
                                  tag="idxhi")
            nc.sync.dma_start(idxhi_sb[:], idxhi_d[:])
            dinvbc_sb = cpool.tile([128, npcp], bf, tag="dinvbc")
            nc.sync.dma_start(dinvbc_sb[:], dinvbc_d[:])
            dinvnm_sb = cpool.tile([128, tpc], f32, tag="dinvnm")
            nc.sync.dma_start(dinvnm_sb[:], dinvnm_d[:])
            ident = cpool.tile([OUT_C, OUT_C], f32, tag="ident")
            make_identity(nc, ident[:])

            ta, spA = cfg.TA, cfg.SPLITA
            szB = npcp - spA
            ag_inA = [dpool.tile([spA, C], bf, name=f"ag_inA{i}")
                      for i in range(3)]
            ag_inB = [dpool.tile([szB, C], bf, name=f"ag_inB{i}")
                      for i in range(3)]
            xt_fullA = [dpool.tile([cfg.ROWSA, C], bf, name=f"xt_fullA{i}")
                        for i in range(3)]
            xt_fullB = [dpool.tile([cfg.ROWSB, C], bf, name=f"xt_fullB{i}")
                        for i in range(3)]

            def emit_ag(li, nm_tile):
                nc.sync.dma_start(
                    ag_inA[li][:].rearrange("(t p) c -> p t c", p=128),
                    nm_tile[:, :ta, :])
                nc.sync.dma_start(
                    ag_inB[li][:].rearrange("(t p) c -> p t c", p=128),
                    nm_tile[:, ta:, :])
                for buf_in, buf_out in ((ag_inA[li], xt_fullA[li]),
                                        (ag_inB[li], xt_fullB[li])):
                    nc.gpsimd.collective_compute(
                        "AllGather", mybir.AluOpType.bypass,
                        replica_groups=[list(range(cfg.NCORES))],
                        ins=[buf_in.opt()], outs=[buf_out.opt()],
                    )

            # ---- prologue: x_tilde(own slice) = dinv * x, then AllGather
            x_own = nmpool.tile([128, tpc, C], bf, tag="nm")
            nc.sync.dma_start(
                x_own[:], xt0_d[:].rearrange("(t p) c -> p t c", p=128))
            xs_own = xtpool.tile([128, tpc, C], bf, tag="xt")
            for t in range(tpc):
                nc.vector.tensor_scalar(
                    xs_own[:, t, :], x_own[:, t, :], dinvnm_sb[:, t:t + 1],
                    None, mybir.AluOpType.mult)
            emit_ag(0, xs_own)

            lo_total, hi_total = sum(clo), sum(chi)

            for layer in range(3):
                cout = C if layer < 2 else OUT_C

                # ---- gathers (lo then hi), round-robin over 4 SWDGE
                # queues = 4 Q7 core pairs generating descriptors in parallel
                g_slots = []
                qrr = 0
                for total, idx_sb, src_dram in (
                        (lo_total, idxlo_sb, xt_fullA[layer][:]),
                        (hi_total, idxhi_sb, xt_fullB[layer][:])):
                    c0 = 0
                    while c0 < total:
                        pch = min(cfg.PIECE_CH, total - c0)
                        g = gpool.tile([128, cfg.PIECE_CH, C], bf, tag="g")
                        nc.gpsimd.dma_gather(
                            g[:, :pch, :],
                            src_dram,
                            idx_sb[:, c0 * 8:(c0 + pch) * 8],
                            pch * 128,
                            pch * 128,
                            C,
                            single_packet=True,
                            queue_num=qrr % 4,
                        )
                        qrr += 1
                        for k in range(pch):
                            g_slots.append((g, k))
                        c0 += pch

                # ---- S blocks streamed from HBM (contiguous layout)
                s_slots = []
                for b in range(nblk_s):
                    s = spool.tile([128, cfg.SBATCH, 128], bf, tag="s")
                    nc.scalar.dma_start(
                        s[:].rearrange("p k n -> p (k n)"),
                        s_d[b * 128:(b + 1) * 128, :])
                    for k in range(cfg.SBATCH):
                        s_slots.append((s, k))
                s_slots = s_slots[:nch]

                # ---- segment-sum matmuls, accumulating z^T per dst tile.
                # z is split into per-512-col block tiles so the zs/W tail
                # of block b only depends on its own four dst tiles.
                nzb = (npcp + 511) // 512
                zb = [zpool.tile([128, 512], f32, tag="z", name=f"zb{i}")
                      for i in range(nzb)]

                def zsl(t):
                    return zb[t // 4][:, (t % 4) * 128:(t % 4) * 128 + 128]

                gi = 0
                for phase, cc in ((0, clo), (1, chi)):
                    for t in range(tpc):
                        cnt = cc[t]
                        if cnt == 0:
                            if phase == 0 and chi[t] == 0:
                                nc.vector.memset(zsl(t), 0.0)
                            continue
                        ps = psa.tile([128, 128], f32, tag="psA")
                        for k in range(cnt):
                            g, gk = g_slots[gi]
                            s, sk = s_slots[gi]
                            gi += 1
                            nc.tensor.matmul(ps[:], g[:, gk, :], s[:, sk, :],
                                             start=(k == 0),
                                             stop=(k == cnt - 1))
                        if phase == 0 or clo[t] == 0:
                            nc.scalar.copy(zsl(t), ps[:])
                        else:
                            nc.vector.tensor_add(zsl(t), zsl(t), ps[:])
                assert gi == nch

                # ---- dinv[dst] column scale, W matmul, bias(/relu)
                if layer < 2:
                    xt = xtpool.tile([128, npcp], bf, tag="xt")
                else:
                    fin = finpool.tile([OUT_C, npcp], f32, tag="fin")

                nblk = [(i * 512, min(512, npcp - i * 512))
                        for i in range((npcp + 511) // 512)]
                for bi, (bo, bs) in enumerate(nblk):
                    sl = np.s_[:, bo:bo + bs]
                    zs = zspool.tile([128, 512], bf, tag="zs")
                    nc.vector.tensor_tensor(zs[:, :bs], zb[bi][:, :bs],
                                            dinvbc_sb[sl],
                                            mybir.AluOpType.mult)
                    psw = psw_pool.tile([cout, 512], f32, tag="psW")
                    nc.tensor.matmul(psw[:, :bs], w_sb[layer][:],
                                     zs[:, :bs], start=True, stop=True)
                    if layer < 2:
                        tmp = zspool.tile([128, 512], bf, tag="acttmp")
                        nc.scalar.activation(
                            tmp[:, :bs], psw[:, :bs],
                            mybir.ActivationFunctionType.Relu,
                            bias=b_sb[layer][:])
                        nc.vector.tensor_tensor(xt[sl], tmp[:, :bs],
                                                dinvbc_sb[sl],
                                                mybir.AluOpType.mult)
                    else:
                        nc.scalar.activation(
                            fin[sl], psw[:cout, :bs],
                            mybir.ActivationFunctionType.Identity,
                            bias=b_sb[layer][:])

                if layer < 2:
                    # node-major transpose + split AllGather
                    xt_nm = nmpool.tile([128, tpc, C], bf, tag="nm")
                    nc.sync.dma_start_transpose(xt_nm[:], xt[:])
                    emit_ag(layer + 1, xt_nm)
                else:
                    # final: transpose 64xN^T -> node-major fp32, DMA out
                    out_nm = finpool.tile([128, tpc, OUT_C], f32, tag="onm")
                    for t in range(tpc):
                        tp = pst.tile([128, OUT_C], f32, tag="psT")
                        nc.tensor.transpose(
                            tp[:], fin[:, t * 128:(t + 1) * 128], ident[:])
                        nc.scalar.copy(out_nm[:, t, :], tp[:])
                    nfull = cfg.NPC // 128
                    rem = cfg.NPC - nfull * 128
                    nc.sync.dma_start(
                        out_d[:nfull * 128].rearrange("(t p) c -> p t c",
                                                      p=128),
                        out_nm[:, :nfull, :])
                    if rem:
                        nc.sync.dma_start(out_d[nfull * 128:cfg.NPC],
                                          out_nm[:rem, nfull, :])

    nc.compile()
    return nc


# ------------------------------------------------------------------ driver
_CACHE = {}


def _get_program(cfg, plan):
    key = (cfg.N, cfg.NCORES, tuple(plan["clo"]), tuple(plan["chi"]))
    if key not in _CACHE:
        _CACHE[key] = _build(cfg, plan)
    return _CACHE[key]


def _make_in_maps(cfg, x, weights, biases, plan, per_core):
    x = np.asarray(x, dtype=np.float32)
    npc, npcp = cfg.NPC, cfg.NPC_PAD

    in_maps = []
    for c in range(cfg.NCORES):
        xt0 = np.zeros((npcp, cfg.C), dtype=BF16)
        xt0[:npc] = x[c * npc:(c + 1) * npc].astype(BF16)
        m = {
            "xt0": xt0,
            "idx_lo": per_core[c]["idx_lo"],
            "idx_hi": per_core[c]["idx_hi"],
            "s_all": per_core[c]["s_all"],
            "dinv_bc": per_core[c]["dinv_bc"],
            "dinv_nm": per_core[c]["dinv_nm"],
        }
        for i in range(3):
            m[f"w{i}"] = np.asarray(weights[i], dtype=np.float32) \
                .astype(BF16)
            m[f"b{i}"] = np.asarray(biases[i], dtype=np.float32) \
                .reshape(-1, 1)
        in_maps.append(m)
    return in_maps


def run(cfg, x, edge_index, weights, biases, sim=False, trace=False):
    plan, per_core, _ = _preprocess(cfg, edge_index)
    nc = _get_program(cfg, plan)
    in_maps = _make_in_maps(cfg, x, weights, biases, plan, per_core)

    if sim:
        from concourse.bass_interp import MultiCoreSim

        s = MultiCoreSim(nc, num_cores=cfg.NCORES, num_workers=1)
        for c in range(cfg.NCORES):
            for k, v in in_maps[c].items():
                s.cores[c].tensor(k)[:] = v
        s.simulate()
        results = [{"out": s.cores[c].tensor("out").copy()}
                   for c in range(cfg.NCORES)]
        res = None
    else:
        _install_ntff_hook()
        res = bass_utils.run_bass_kernel_spmd(
            nc, in_maps, core_ids=list(range(cfg.NCORES)), trace=trace)
        results = res.results

    out = np.concatenate([results[c]["out"] for c in range(cfg.NCORES)], 0)
    return out, res


def kernel(x, edge_index, W1, b1, W2, b2, W3, b3):
    out, _ = run(FULL, x, edge_index, (W1, W2, W3), (b1, b2, b3))
    return out


# revision 25
# speedup vs baseline: 1.0101x; 1.0101x over previous
"""3-layer GCN forward pass on 8 TRN2 NeuronCores.

Strategy (vertex-cut graph parallelism):
  - Each core owns a contiguous block of N/8 destination nodes; edges are
    partitioned by destination so segment sums stay local.
  - Per layer, each GCN conv is computed as (A_hat @ x) @ W + b, i.e.
    aggregate first, then the dense matmul.
  - Prologue: x_tilde = dinv * x is computed per-core on its own slice and
    AllGathered; every layer then gathers x_tilde rows per edge with
    dma_gather (4 SWDGE queues in parallel) and segment-sums them on the
    tensor engine as one-hot matmuls accumulating z^T in PSUM (channels on
    partitions).  One-hot S matrices depend only on the graph; they are
    precomputed host-side and streamed from HBM.
  - The destination-side dinv is applied per-column after aggregation;
    self loops are plain edges, reproducing the h/deg self term exactly.
  - Between layers: the per-core slice of the new features is scaled,
    transposed back to node-major (DMA transpose) and AllGathered.
  - int16 gather indices only reach 32768 rows, so each core's edges are
    split into a "lo" stream (padded row id < 32768) and a "hi" stream,
    gathered from two base offsets; PSUM partials from the two passes are
    combined in SBUF.
"""

import sys
import types

sys.path.insert(0, "/opt/trn_rl_repo")

import numpy as np
import ml_dtypes

import concourse.bass as bass  # noqa: F401
import concourse.bacc as bacc
import concourse.tile as tile
import concourse.mybir as mybir
from concourse import bass_utils
from concourse.masks import make_identity

BF16 = ml_dtypes.bfloat16
FP32 = np.float32


def _install_ntff_hook():
    """The image's antenv lacks axon_hooks; shim it so trace=True works."""
    if "antenv.axon_hooks" in sys.modules:
        return
    mod = types.ModuleType("antenv.axon_hooks")
    mod._hook = None
    mod.set_axon_ntff_profile_hook = lambda h: setattr(mod, "_hook", h)
    mod.get_axon_ntff_profile_hook = lambda: mod._hook
    sys.modules["antenv.axon_hooks"] = mod
    try:
        import antenv

        antenv.axon_hooks = mod
        if "/root/.axon_site" not in sys.path:
            sys.path.insert(0, "/root/.axon_site")
        from trn_agent_boot.trn_boot import _ntff_profile_via_ctypes

        mod.set_axon_ntff_profile_hook(
            _ntff_profile_via_ctypes("/opt/axon/libaxon_pjrt.so"))
    except Exception:
        pass


class Cfg:
    def __init__(self, n=50000, c=128, hid=128, out_c=64, ncores=8,
                 lo_rows=32768, piece_ch=8, sbatch=16):
        self.N = n
        self.C = c
        self.HID = hid
        self.OUT_C = out_c
        self.NCORES = ncores
        self.NPC = n // ncores
        self.TPC = (self.NPC + 127) // 128
        self.NPC_PAD = self.TPC * 128
        self.NPAD = ncores * self.NPC_PAD
        self.LO_ROWS = lo_rows
        self.TA = (self.TPC + 1) // 2          # tiles in half A
        self.SPLITA = self.TA * 128            # per-rank rows in half A
        self.ROWSA = ncores * self.SPLITA
        self.ROWSB = ncores * (self.NPC_PAD - self.SPLITA)
        assert self.ROWSA <= 32768 and self.ROWSB <= 32768, \
            "half-buffers must stay int16-addressable"
        self.PIECE_CH = piece_ch
        self.SBATCH = sbatch
        assert n % ncores == 0


FULL = Cfg()


# ---------------------------------------------------------------- host prep
def _preprocess(cfg, edge_index):
    """Compute per-core gather indices and one-hot S blocks."""
    src = np.asarray(edge_index[0]).astype(np.int64)
    dst = np.asarray(edge_index[1]).astype(np.int64)
    n, npc, npcp, tpc = cfg.N, cfg.NPC, cfg.NPC_PAD, cfg.TPC

    deg = (np.bincount(dst, minlength=n) + 1.0).astype(np.float64)
    dinv = (1.0 / np.sqrt(deg)).astype(np.float32)

    loop = np.arange(n, dtype=np.int64)
    allsrc = np.concatenate([src, loop])
    alldst = np.concatenate([dst, loop])

    src_r = allsrc // npc
    src_i = allsrc % npc
    bucket = (src_i >= cfg.SPLITA).astype(np.int64)
    szB = npcp - cfg.SPLITA
    m = np.where(bucket == 0, src_r * cfg.SPLITA + src_i,
                 src_r * szB + (src_i - cfg.SPLITA))
    owner = alldst // npc
    local = alldst - owner * npc
    tile_id = local // 128
    pos = local % 128

    order = np.lexsort((m, bucket, tile_id, owner))
    m_s = m[order]
    owner_s = owner[order]
    tile_s = tile_id[order]
    bucket_s = bucket[order]
    pos_s = pos[order]

    counts = np.zeros((cfg.NCORES, tpc, 2), dtype=np.int64)
    np.add.at(counts, (owner_s, tile_s, bucket_s), 1)
    clo = np.ceil(counts[:, :, 0] / 128).astype(int).max(axis=0)  # [tpc]
    chi = np.ceil(counts[:, :, 1] / 128).astype(int).max(axis=0)

    flat_counts = counts.reshape(-1)
    flat_starts = np.concatenate([[0], np.cumsum(flat_counts)[:-1]])
    starts = flat_starts.reshape(cfg.NCORES, tpc, 2)

    nch = int(clo.sum() + chi.sum())
    l_lo = max(int(clo.sum()) * 128, 16)
    l_hi = max(int(chi.sum()) * 128, 16)

    col_ids = np.arange(128, dtype=np.int32)
    per_core = []
    for c in range(cfg.NCORES):
        idx_lo = np.zeros(l_lo, dtype=np.int32)
        idx_hi = np.zeros(l_hi, dtype=np.int32)
        dpos = np.full((nch, 128), 255, dtype=np.int32)

        ch_g = 0
        for b, cc, idx_arr in ((0, clo, idx_lo), (1, chi, idx_hi)):
            off = 0
            for t in range(tpc):
                s0 = starts[c, t, b]
                cnt = counts[c, t, b]
                cap = cc[t] * 128
                idx_arr[off:off + cnt] = m_s[s0:s0 + cnt]
                blk = dpos[ch_g:ch_g + cc[t]].reshape(-1)
                blk[:cnt] = pos_s[s0:s0 + cnt]
                off += cap
                ch_g += cc[t]
        assert ch_g == nch

        def wrap(stream):
            # idx j consumed from [j % 16, j // 16]; replicate to 8 groups
            a = stream.reshape(-1, 16).T.astype(np.int16)
            return np.tile(a, (8, 1)).copy()

        nblk_s = (nch + cfg.SBATCH - 1) // cfg.SBATCH
        dpos_pad = np.full((nblk_s * cfg.SBATCH, 128), 255, dtype=np.int32)
        dpos_pad[:nch] = dpos
        s_all = (dpos_pad[:, :, None] == col_ids[None, None, :]).astype(BF16)
        # block-contiguous layout: [nblk, 128 part, SBATCH, 128]
        s_all = s_all.reshape(nblk_s, cfg.SBATCH, 128, 128) \
            .transpose(0, 2, 1, 3).reshape(nblk_s * 128, cfg.SBATCH * 128)

        dinv_own = np.zeros(npcp, dtype=np.float32)
        dinv_own[:npc] = dinv[c * npc:(c + 1) * npc]
        per_core.append({
            "idx_lo": wrap(idx_lo),
            "idx_hi": wrap(idx_hi),
            "s_all": np.ascontiguousarray(s_all),
            "dinv_bc": np.tile(dinv_own, (128, 1)).astype(BF16),
            "dinv_nm": np.ascontiguousarray(
                dinv_own.reshape(tpc, 128).T).astype(np.float32),
        })

    plan = {
        "clo": clo.tolist(),
        "chi": chi.tolist(),
        "nch": nch,
        "nblk_s": nblk_s,
        "l_lo": l_lo,
        "l_hi": l_hi,
    }
    return plan, per_core, dinv


# ------------------------------------------------------------- bass program
def _build(cfg, plan):
    clo, chi = plan["clo"], plan["chi"]
    nch, l_lo, l_hi = plan["nch"], plan["l_lo"], plan["l_hi"]
    tpc, npcp, npad = cfg.TPC, cfg.NPC_PAD, cfg.NPAD
    C, OUT_C = cfg.C, cfg.OUT_C
    bf = mybir.dt.bfloat16
    f32 = mybir.dt.float32

    nc = bacc.Bacc("TRN2", target_bir_lowering=False, debug=False,
                   num_devices=cfg.NCORES, num_swdge_queues=4,
                   dynamic_dma_scratch_size=49152)

    xt0_d = nc.dram_tensor("xt0", [npcp, C], bf, kind="ExternalInput")
    w_d = [nc.dram_tensor(f"w{i}", [C, C if i < 2 else OUT_C], bf,
                          kind="ExternalInput") for i in range(3)]
    b_d = [nc.dram_tensor(f"b{i}", [C if i < 2 else OUT_C, 1], f32,
                          kind="ExternalInput") for i in range(3)]
    idxlo_d = nc.dram_tensor("idx_lo", [128, max(l_lo // 16, 1)],
                             mybir.dt.int16, kind="ExternalInput")
    idxhi_d = nc.dram_tensor("idx_hi", [128, max(l_hi // 16, 1)],
                             mybir.dt.int16, kind="ExternalInput")
    nblk_s = plan["nblk_s"]
    s_d = nc.dram_tensor("s_all", [nblk_s * 128, cfg.SBATCH * 128], bf,
                         kind="ExternalInput")
    dinvbc_d = nc.dram_tensor("dinv_bc", [128, npcp], bf,
                              kind="ExternalInput")
    dinvnm_d = nc.dram_tensor("dinv_nm", [128, tpc], f32,
                              kind="ExternalInput")
    out_d = nc.dram_tensor("out", [cfg.NPC, OUT_C], f32,
                           kind="ExternalOutput")

    with tile.TileContext(nc) as tc:
        with (
            tc.tile_pool(name="const", bufs=1) as cpool,
            tc.tile_pool(name="g", bufs=8) as gpool,
            tc.tile_pool(name="s", bufs=5) as spool,
            tc.tile_pool(name="z", bufs=13) as zpool,
            tc.tile_pool(name="zs", bufs=2) as zspool,
            tc.tile_pool(name="xt", bufs=1) as xtpool,
            tc.tile_pool(name="nm", bufs=1) as nmpool,
            tc.tile_pool(name="fin", bufs=1) as finpool,
            tc.tile_pool(name="psA", bufs=5, space="PSUM") as psa,
            tc.tile_pool(name="psW", bufs=2, space="PSUM") as psw_pool,
            tc.tile_pool(name="psT", bufs=1, space="PSUM") as pst,
            tc.tile_pool(name="dram", bufs=1, space="DRAM") as dpool,
        ):
            # ---- constants into SBUF
            w_sb, b_sb = [], []
            for i in range(3):
                w = cpool.tile([C, C if i < 2 else OUT_C], bf, name=f"wt{i}")
                nc.sync.dma_start(w[:], w_d[i][:])
                bt = cpool.tile([C if i < 2 else OUT_C, 1], f32,
                                name=f"bt{i}")
                nc.sync.dma_start(bt[:], b_d[i][:])
                w_sb.append(w)
                b_sb.append(bt)
            idxlo_sb = cpool.tile([128, max(l_lo // 16, 1)], mybir.dt.int16,
                                  tag="idxlo")
            nc.sync.dma_start(idxlo_sb[:], idxlo_d[:])
            idxhi_sb = cpool.tile([128, max(l_hi // 16, 1)], mybir.dt.int16,
                                  tag="idxhi")
            nc.sync.dma_start(idxhi_sb[:], idxhi_d[:])
            dinvbc_sb = cpool.tile([128, npcp], bf, tag="dinvbc")
            nc.sync.dma_start(dinvbc_sb[:], dinvbc_d[:])
            dinvnm_sb = cpool.tile([128, tpc], f32, tag="dinvnm")
            nc.sync.dma_start(dinvnm_sb[:], dinvnm_d[:])
            ident = cpool.tile([OUT_C, OUT_C], f32, tag="ident")
            make_identity(nc, ident[:])

            ta, spA = cfg.TA, cfg.SPLITA
            szB = npcp - spA
            ag_inA = [dpool.tile([spA, C], bf, name=f"ag_inA{i}")
                      for i in range(3)]
            ag_inB = [dpool.tile([szB, C], bf, name=f"ag_inB{i}")
                      for i in range(3)]
            xt_fullA = [dpool.tile([cfg.ROWSA, C], bf, name=f"xt_fullA{i}")
                        for i in range(3)]
            xt_fullB = [dpool.tile([cfg.ROWSB, C], bf, name=f"xt_fullB{i}")
                        for i in range(3)]

            def emit_ag(li, nm_tile):
                nc.sync.dma_start(
                    ag_inA[li][:].rearrange("(t p) c -> p t c", p=128),
                    nm_tile[:, :ta, :])
                nc.sync.dma_start(
                    ag_inB[li][:].rearrange("(t p) c -> p t c", p=128),
                    nm_tile[:, ta:, :])
                for buf_in, buf_out in ((ag_inA[li], xt_fullA[li]),
                                        (ag_inB[li], xt_fullB[li])):
                    nc.gpsimd.collective_compute(
                        "AllGather", mybir.AluOpType.bypass,
                        replica_groups=[list(range(cfg.NCORES))],
                        ins=[buf_in.opt()], outs=[buf_out.opt()],
                    )

            # ---- prologue: x_tilde(own slice) = dinv * x, then AllGather
            x_own = nmpool.tile([128, tpc, C], bf, tag="nm")
            nc.sync.dma_start(
                x_own[:], xt0_d[:].rearrange("(t p) c -> p t c", p=128))
            xs_own = xtpool.tile([128, tpc, C], bf, tag="xt")
            for t in range(tpc):
                nc.vector.tensor_scalar(
                    xs_own[:, t, :], x_own[:, t, :], dinvnm_sb[:, t:t + 1],
                    None, mybir.AluOpType.mult)
            emit_ag(0, xs_own)

            lo_total, hi_total = sum(clo), sum(chi)

            for layer in range(3):
                cout = C if layer < 2 else OUT_C

                # ---- gathers (lo then hi), round-robin over 4 SWDGE
                # queues = 4 Q7 core pairs generating descriptors in parallel
                g_slots = []
                qrr = 0
                for total, idx_sb, src_dram in (
                        (lo_total, idxlo_sb, xt_fullA[layer][:]),
                        (hi_total, idxhi_sb, xt_fullB[layer][:])):
                    c0 = 0
                    while c0 < total:
                        pch = min(cfg.PIECE_CH, total - c0)
                        g = gpool.tile([128, cfg.PIECE_CH, C], bf, tag="g")
                        nc.gpsimd.dma_gather(
                            g[:, :pch, :],
                            src_dram,
                            idx_sb[:, c0 * 8:(c0 + pch) * 8],
                            pch * 128,
                            pch * 128,
                            C,
                            single_packet=True,
                            queue_num=qrr % 4,
                        )
                        qrr += 1
                        for k in range(pch):
                            g_slots.append((g, k))
                        c0 += pch

                # ---- S blocks streamed from HBM (contiguous layout)
                s_slots = []
                for b in range(nblk_s):
                    s = spool.tile([128, cfg.SBATCH, 128], bf, tag="s")
                    nc.scalar.dma_start(
                        s[:].rearrange("p k n -> p (k n)"),
                        s_d[b * 128:(b + 1) * 128, :])
                    for k in range(cfg.SBATCH):
                        s_slots.append((s, k))
                s_slots = s_slots[:nch]

                # ---- segment-sum matmuls, accumulating z^T per dst tile.
                # z is split into per-512-col block tiles so the zs/W tail
                # of block b only depends on its own four dst tiles.
                nzb = (npcp + 511) // 512
                zb = [zpool.tile([128, 512], f32, tag="z", name=f"zb{i}")
                      for i in range(nzb)]

                def zsl(t):
                    return zb[t // 4][:, (t % 4) * 128:(t % 4) * 128 + 128]

                gi = 0
                for phase, cc in ((0, clo), (1, chi)):
                    for t in range(tpc):
                        cnt = cc[t]
                        if cnt == 0:
                            if phase == 0 and chi[t] == 0:
                                nc.vector.memset(zsl(t), 0.0)
                            continue
                        ps = psa.tile([128, 128], f32, tag="psA")
                        for k in range(cnt):
                            g, gk = g_slots[gi]
                            s, sk = s_slots[gi]
                            gi += 1
                            nc.tensor.matmul(ps[:], g[:, gk, :], s[:, sk, :],
                                             start=(k == 0),
                                             stop=(k == cnt - 1))
                        if phase == 0 or clo[t] == 0:
                            nc.scalar.copy(zsl(t), ps[:])
                        else:
                            nc.vector.tensor_add(zsl(t), zsl(t), ps[:])
                assert gi == nch

                # ---- dinv[dst] column scale, W matmul, bias(/relu)
                if layer < 2:
                    xt = xtpool.tile([128, npcp], bf, tag="xt")
                else:
                    fin = finpool.tile([OUT_C, npcp], f32, tag="fin")

                nblk = [(i * 512, min(512, npcp - i * 512))
                        for i in range((npcp + 511) // 512)]
                for bi, (bo, bs) in enumerate(nblk):
                    sl = np.s_[:, bo:bo + bs]
                    zs = zspool.tile([128, 512], bf, tag="zs")
                    nc.vector.tensor_tensor(zs[:, :bs], zb[bi][:, :bs],
                                            dinvbc_sb[sl],
                                            mybir.AluOpType.mult)
                    psw = psw_pool.tile([cout, 512], f32, tag="psW")
                    nc.tensor.matmul(psw[:, :bs], w_sb[layer][:],
                                     zs[:, :bs], start=True, stop=True)
                    if layer < 2:
                        tmp = zspool.tile([128, 512], bf, tag="acttmp")
                        nc.scalar.activation(
                            tmp[:, :bs], psw[:, :bs],
                            mybir.ActivationFunctionType.Relu,
                            bias=b_sb[layer][:])
                        nc.vector.tensor_tensor(xt[sl], tmp[:, :bs],
                                                dinvbc_sb[sl],
                                                mybir.AluOpType.mult)
                    else:
                        nc.scalar.activation(
                            fin[sl], psw[:cout, :bs],
                            mybir.ActivationFunctionType.Identity,
                            bias=b_sb[layer][:])

                if layer < 2:
                    # node-major transpose + split AllGather
                    xt_nm = nmpool.tile([128, tpc, C], bf, tag="nm")
                    nc.sync.dma_start_transpose(xt_nm[:], xt[:])
                    emit_ag(layer + 1, xt_nm)
                else:
                    # final: transpose 64xN^T -> node-major fp32, DMA out
                    out_nm = finpool.tile([128, tpc, OUT_C], f32, tag="onm")
                    for t in range(tpc):
                        tp = pst.tile([128, OUT_C], f32, tag="psT")
                        nc.tensor.transpose(
                            tp[:], fin[:, t * 128:(t + 1) * 128], ident[:])
                        nc.scalar.copy(out_nm[:, t, :], tp[:])
                    nfull = cfg.NPC // 128
                    rem = cfg.NPC - nfull * 128
                    nc.sync.dma_start(
                        out_d[:nfull * 128].rearrange("(t p) c -> p t c",
                                                      p=128),
                        out_nm[:, :nfull, :])
                    if rem:
                        nc.sync.dma_start(out_d[nfull * 128:cfg.NPC],
                                          out_nm[:rem, nfull, :])

    nc.compile()
    return nc


# ------------------------------------------------------------------ driver
_CACHE = {}


def _get_program(cfg, plan):
    key = (cfg.N, cfg.NCORES, tuple(plan["clo"]), tuple(plan["chi"]))
    if key not in _CACHE:
        _CACHE[key] = _build(cfg, plan)
    return _CACHE[key]


def _make_in_maps(cfg, x, weights, biases, plan, per_core):
    x = np.asarray(x, dtype=np.float32)
    npc, npcp = cfg.NPC, cfg.NPC_PAD

    in_maps = []
    for c in range(cfg.NCORES):
        xt0 = np.zeros((npcp, cfg.C), dtype=BF16)
        xt0[:npc] = x[c * npc:(c + 1) * npc].astype(BF16)
        m = {
            "xt0": xt0,
            "idx_lo": per_core[c]["idx_lo"],
            "idx_hi": per_core[c]["idx_hi"],
            "s_all": per_core[c]["s_all"],
            "dinv_bc": per_core[c]["dinv_bc"],
            "dinv_nm": per_core[c]["dinv_nm"],
        }
        for i in range(3):
            m[f"w{i}"] = np.asarray(weights[i], dtype=np.float32) \
                .astype(BF16)
            m[f"b{i}"] = np.asarray(biases[i], dtype=np.float32) \
                .reshape(-1, 1)
        in_maps.append(m)
    return in_maps


def run(cfg, x, edge_index, weights, biases, sim=False, trace=False):
    plan, per_core, _ = _preprocess(cfg, edge_index)
    nc = _get_program(cfg, plan)
    in_maps = _make_in_maps(cfg, x, weights, biases, plan, per_core)

    if sim:
        from concourse.bass_interp import MultiCoreSim

        s = MultiCoreSim(nc, num_cores=cfg.NCORES, num_workers=1)
        for c in range(cfg.NCORES):
            for k, v in in_maps[c].items():
                s.cores[c].tensor(k)[:] = v
        s.simulate()
        results = [{"out": s.cores[c].tensor("out").copy()}
                   for c in range(cfg.NCORES)]
        res = None
    else:
        _install_ntff_hook()
        res = bass_utils.run_bass_kernel_spmd(
            nc, in_maps, core_ids=list(range(cfg.NCORES)), trace=trace)
        results = res.results

    out = np.concatenate([results[c]["out"] for c in range(cfg.NCORES)], 0)
    return out, res


def kernel(x, edge_index, W1, b1, W2, b2, W3, b3):
    out, _ = run(FULL, x, edge_index, (W1, W2, W3), (b1, b2, b3))
    return out
